# revision 6
# baseline (speedup 1.0000x reference)
"""Trainium2 Bass kernel for nn_DirectedGNNLayer (bipartite GATv2 x2), v2.

Design (vs v1): host pre-gathers and pre-transposes the raw source
features per edge slot into [D, 128]-column tiles, so the kernel needs
NO on-device gather, NO dedup table, and NO xl DRAM round trip:

  * per 128-slot block (supertile column group), PE computes
      psumZ = x_src@Wl + x_dst@Wr + ew (x) We     (3 accumulating mms)
      psumX = x_src@Wl_m                          (xl for the msg path)
  * Act does Prelu(psumZ) -> zp (bf16), Pool copies psumX -> xlg (bf16)
  * channels are c-major (hc = c*H + h) so every big DVE op is packed
    bf16 (2x mode): att-mult, treeC, msg-mult, treeW
  * softmax over w per (p, nb) node row as in v1; output ELU as in v1.

kernel(**inputs) takes FULL inputs, returns FULL (s_out, t_out).
"""
import sys
import os
import numpy as np
import ml_dtypes

sys.path.insert(0, '/opt/trn_rl_repo')

BF = ml_dtypes.bfloat16

N = 100000
D = 128
E = 800000
H = 4
C = 32
HC = H * C
NEG = 0.2
P = 128
NCORES = 8
CAP = 20
NBMAX = 8

# c-major channel permutation: new index c*H + h  <-  old index h*C + c
_OLD_OF_NEW = np.array([(i % H) * C + (i // H) for i in range(HC)], np.int64)
_NEW_OF_OLD = np.argsort(_OLD_OF_NEW)


def _patch_walrus():
    from concourse import bass_utils
    if getattr(bass_utils, "_ant_dge_patched", None) == "v2":
        return
    orig = getattr(bass_utils, "_ant_orig_walrus_args", None) \
        or bass_utils.get_walrus_args
    bass_utils._ant_orig_walrus_args = orig

    def patched(*a, **k):
        return orig(*a, **k) + [
            "--dge-levels=io,scalar_dynamic_offset,vector_dynamic_offsets",
        ]

    bass_utils.get_walrus_args = patched
    bass_utils._ant_dge_patched = "v2"


def _encoder_prep(n_nodes, x_src, x_dst, src, dst, edge_w, n_cores):
    """Geometry + per-core host arrays for one encoder (v2 packing).

    Partitions target (dst) nodes round-robin by degree rank; supertiles
    of P rows x NB nodes x W slots.  Host packs, per core:
      colT [D, 128*G]  src features, block B=(t,nb,w): col 128*B+p
      dstT [D, 128*TB] dst features, block (t,nb): col 128*(dstO+nb)+p
      ewT  [1, 128*G]  edge weight per slot
      mask [P, G]      0 real / -1e30 pad
    """
    ne = len(dst)
    deg = np.bincount(dst, minlength=n_nodes)
    order = np.argsort(-deg, kind='stable')
    order = order[deg[order] > 0]
    K = len(order)

    core_of = np.full(n_nodes, -1, np.int32)
    pos_of = np.full(n_nodes, -1, np.int64)
    idx = np.arange(K)
    core_of[order] = (idx % n_cores).astype(np.int32)
    pos_of[order] = idx // n_cores
    n_loc = (K + n_cores - 1) // n_cores

    deg_rank = deg[order[0::n_cores]]

    Ws, NBs, starts = [], [], []
    pos = 0
    while pos < n_loc:
        W = int(deg_rank[pos]) if pos < len(deg_rank) else 1
        W = max(W, 1)
        NB = max(1, min(NBMAX, CAP // W))
        starts.append(pos)
        Ws.append(W)
        NBs.append(NB)
        pos += P * NB
    n_loc_pad = pos
    S = len(Ws)
    colO = np.zeros(S + 1, np.int64)
    dstO = np.zeros(S + 1, np.int64)
    for t in range(S):
        colO[t + 1] = colO[t] + NBs[t] * Ws[t]
        dstO[t + 1] = dstO[t] + NBs[t]
    G = int(colO[-1])
    TB = int(dstO[-1])

    # local index j = starts[t] + p*NB + nb  <->  (t, p, nb)
    row_of = np.empty(n_loc_pad, np.int64)
    colb_of = np.empty(n_loc_pad, np.int64)
    for t in range(S):
        r = np.arange(P * NBs[t])
        sl = slice(starts[t], starts[t] + P * NBs[t])
        row_of[sl] = r // NBs[t]
        colb_of[sl] = colO[t] + (r % NBs[t]) * Ws[t]

    # within-node slot counter w for each edge
    sidx = np.argsort(dst, kind='stable')
    sdst = dst[sidx]
    first = np.r_[True, sdst[1:] != sdst[:-1]]
    run_starts_pos = np.flatnonzero(first)
    run_id = np.cumsum(first) - 1
    w_sorted = np.arange(ne) - run_starts_pos[run_id]
    w_of = np.empty(ne, np.int64)
    w_of[sidx] = w_sorted

    c_e = core_of[dst]
    j_e = pos_of[dst]
    row_e = row_of[j_e]          # p
    col_e = colb_of[j_e] + w_of  # slot/block index in [0, G)

    xsrcT = np.ascontiguousarray(x_src.T).astype(BF)
    xdstT = np.ascontiguousarray(x_dst.T).astype(BF)

    colT = np.zeros((n_cores, D, 128 * G), BF)
    ewT = np.zeros((n_cores, 2, 128 * G), BF)
    ewT[:, 1, :] = 1.0
    mask = np.full((n_cores, P, G), -1e30, BF)

    flat = col_e * 128 + row_e
    for c in range(n_cores):
        m = c_e == c
        colT[c][:, flat[m]] = xsrcT[:, src[m]]
        ewT[c][0, flat[m]] = edge_w[m].astype(BF)
    mask[c_e, row_e, col_e] = 0.0

    # combined per-supertile aux stream: [colT-seg | dstT-seg | mask-seg]
    auxO = np.zeros(S + 1, np.int64)
    for t in range(S):
        auxO[t + 1] = auxO[t] + (NBs[t] * Ws[t]) * 128 + NBs[t] * 128 + \
            NBs[t] * Ws[t]
    AW = int(auxO[-1])

    # dst feature blocks + node lists
    dstT = np.zeros((n_cores, D, 128 * TB), BF)
    node_lists = []
    for c in range(n_cores):
        nl = order[c::n_cores]
        node_lists.append(nl)
        nl_pad = np.zeros(n_loc_pad, np.int64)
        nl_pad[:len(nl)] = nl
        valid = np.zeros(n_loc_pad, bool)
        valid[:len(nl)] = True
        for t in range(S):
            NB, W = NBs[t], Ws[t]
            r = np.arange(P * NB)
            j = starts[t] + r
            pp = r // NB
            nb = r % NB
            cols = 128 * (int(dstO[t]) + nb) + pp
            v = valid[j]
            dstT[c][:, cols[v]] = xdstT[:, nl_pad[j][v]]

    aux = np.zeros((n_cores, P, AW), BF)
    for c in range(n_cores):
        for t in range(S):
            NW = NBs[t] * Ws[t]
            NB = NBs[t]
            o = int(auxO[t])
            cO, dO = int(colO[t]), int(dstO[t])
            aux[c][:, o:o + NW * 128] = colT[c][:, 128 * cO:128 * (cO + NW)]
            o += NW * 128
            aux[c][:, o:o + NB * 128] = dstT[c][:, 128 * dO:128 * (dO + NB)]
            o += NB * 128
            aux[c][:, o:o + NW] = mask[c][:, cO:cO + NW]

    return dict(
        S=S, Ws=Ws, NBs=NBs, starts=starts, colO=colO, dstO=dstO,
        G=G, TB=TB, AW=AW, auxO=auxO, n_loc_pad=n_loc_pad,
        aux=aux, ewT=ewT, node_lists=node_lists,
    )


def _b(tile_ap, off, dims):
    """Build a broadcast/strided AP on a tile: partition dim + free dims."""
    import concourse.bass as bass
    return bass.AP(tile_ap.tensor, tile_ap.offset + off,
                   [list(tile_ap.ap[0])] + [list(d) for d in dims])


def _dedupe_ldweights(m):
    """Remove back-to-back InstLdweights with identical weights.

    The PE array keeps the stationary weights across matmuls
    (ldweights=False), so a reload of the exact same AP with no
    semaphore waits/updates is pure overhead (128 row-cycles each).
    Only plain InstMatmult may sit between the dup and its ancestor.
    """
    import concourse.mybir as mybir
    PE = mybir.EngineType.PE
    n_removed = 0
    for fn in m.functions:
        for bb in fn.blocks:
            prev_sig = None
            dead = []
            for inst in bb.instructions:
                t = type(inst).__name__
                if getattr(inst, 'engine', None) != PE:
                    continue
                if t == 'InstLdweights':
                    si = inst.sync_info() if callable(inst.sync_info) \
                        else inst.sync_info
                    clean = si is None or (not si.on_wait and not si.on_update)
                    sig = str(inst.ins[0])
                    if clean and sig == prev_sig and not inst.is_transpose:
                        dead.append(inst)
                    else:
                        prev_sig = sig
                elif t == 'InstMatmult':
                    if inst.is_transpose:
                        prev_sig = None
                else:
                    prev_sig = None
            for inst in dead:
                bb.instructions.remove(inst)
            n_removed += len(dead)
    return n_removed


def _build_program(geos, zero_bias=False, act_prelu=True, loop_reps=1,
                   phase='all'):
    import concourse.mybir as mybir
    import concourse.bacc as bacc
    import concourse.tile as tile

    f32 = mybir.dt.float32
    bf16 = mybir.dt.bfloat16
    AL = mybir.AluOpType
    AF = mybir.ActivationFunctionType
    AX = mybir.AxisListType

    nc = bacc.Bacc("TRN2", target_bir_lowering=False, debug=False)

    dram_in = {}

    def inp(name, shape, dt=bf16):
        t = nc.dram_tensor(name, shape, dt, kind="ExternalInput")
        dram_in[name] = t
        return t

    enc_io = []
    for e, sfx in enumerate("st"):
        geo = geos[e]
        io = dict(
            aux=inp(f"aux_{sfx}", [P, geo["AW"]]),
            ewT=inp(f"ewT_{sfx}", [2, 128 * geo["G"]]),
            wl2=inp(f"wl2_{sfx}", [P, 2 * HC]),
            wr2=inp(f"wr2_{sfx}", [P, 2 * HC]),
            web=inp(f"web_{sfx}", [2, HC]),
            blb=inp(f"blb_{sfx}", [2, HC]),
            attb=inp(f"attb_{sfx}", [P, HC]),
            bb=inp(f"bb_{sfx}", [P, HC]),
            out=nc.dram_tensor(f"out_{sfx}", [geo["n_loc_pad"], HC], f32,
                               kind="ExternalOutput"),
        )
        enc_io.append(io)

    import contextlib
    NWMAX = max(geos[e]["NBs"][i] * geos[e]["Ws"][i]
                for e in range(2) for i in range(geos[e]["S"]))

    with tile.TileContext(nc) as tc:
        with (
            tc.tile_pool(name="const", bufs=1) as cpool,
            tc.tile_pool(name="auxp", bufs=4) as auxp,
            tc.tile_pool(name="ewp", bufs=3) as ewp,
            tc.tile_pool(name="pu", bufs=3, space="PSUM") as pupool,
            tc.tile_pool(name="zp", bufs=4) as zpool,
            tc.tile_pool(name="xlp", bufs=4) as xlpool,
            tc.tile_pool(name="smp", bufs=9) as smp,
            tc.tile_pool(name="outp", bufs=6) as outp,
        ):
            _ls = contextlib.ExitStack()
            if loop_reps > 1:
                _ls.enter_context(tc.For_i(0, loop_reps, 1))

            enc_ct = []
            for e in range(2):
                io = enc_io[e]
                ct = {}
                for nm, pdim, fdim in (("wl2", P, 2 * HC), ("wr2", P, 2 * HC),
                                       ("web", 2, HC), ("blb", 2, HC),
                                       ("attb", P, HC), ("bb", P, HC)):
                    t = cpool.tile([pdim, fdim], bf16, tag=f"{nm}{e}")
                    nc.sync.dma_start(out=t[:], in_=dram_in[f"{nm}_" + "st"[e]].ap())
                    ct[nm] = t
                enc_ct.append(ct)

            for e in range(2):
                io = enc_io[e]
                geo = geos[e]
                ct = enc_ct[e]
                S, Ws, NBs = geo["S"], geo["Ws"], geo["NBs"]
                colO, dstO = geo["colO"], geo["dstO"]

                for t in range(S):
                    W, NB = Ws[t], NBs[t]
                    NW = NB * W
                    FW = NW * HC
                    cO = int(colO[t])
                    aO = int(geo["auxO"][t])
                    aW = NW * 128 + NB * 128 + NW

                    auxt = auxp.tile([P, NWMAX * 128 + NBMAX * 128 + NWMAX],
                                     bf16, tag="aux")
                    nc.sync.dma_start(
                        out=auxt[:, :aW],
                        in_=io["aux"].ap()[:, aO:aO + aW])
                    colt = auxt
                    dsto = NW * 128
                    mko = NW * 128 + NB * 128
                    ewt = ewp.tile([2, NWMAX * 128], bf16, tag="ew")
                    nc.sync.dma_start(
                        out=ewt[:, :NW * 128],
                        in_=io["ewT"].ap()[:, 128 * cO:128 * (cO + NW)])

                    zp = zpool.tile([P, NWMAX * HC], bf16, tag="zp")
                    xlg = xlpool.tile([P, NWMAX * HC], bf16, tag="xlg")

                    # process blocks (nb, w) in chunks of 4; psum layout per
                    # block k: [k*256 : k*256+128] = z, [+128 : +256] = xl.
                    # 4KB tile = 2 zero regions (2 blocks each).
                    for ch in range(0, NW, 4):
                        cnt = min(4, NW - ch)
                        pu = pupool.tile([P, 4 * 2 * HC], f32, tag="pu")
                        # colT mms first (z+xl in one N=256 mm per block)
                        for k in range(cnt):
                            blk = ch + k
                            cs = slice(blk * 128, (blk + 1) * 128)
                            nc.tensor.matmul(
                                out=pu[:, k * 256:(k + 1) * 256],
                                lhsT=colt[:, cs], rhs=ct["wl2"][:],
                                start=(k % 2 == 0), stop=False,
                                skip_group_check=True)
                        # dst mms (same lhsT for w-consecutive blocks)
                        for k in range(cnt):
                            nb = (ch + k) // W
                            nc.tensor.matmul(
                                out=pu[:, k * 256:k * 256 + HC],
                                lhsT=auxt[:, dsto + nb * 128:
                                          dsto + (nb + 1) * 128],
                                rhs=ct["wr2"][:, :HC], start=False,
                                stop=False, skip_group_check=True)
                        # ew/bias rank-2 mms
                        for k in range(cnt):
                            blk = ch + k
                            cs = slice(blk * 128, (blk + 1) * 128)
                            lastr = (k % 2 == 1) or (k == cnt - 1)
                            nc.tensor.matmul(
                                out=pu[:, k * 256:k * 256 + HC],
                                lhsT=ewt[:, cs], rhs=ct["web"][:],
                                start=False, stop=(lastr and zero_bias),
                                skip_group_check=True)
                            if not zero_bias:
                                nc.tensor.matmul(
                                    out=pu[:, k * 256 + HC:(k + 1) * 256],
                                    lhsT=ewt[:, cs], rhs=ct["blb"][:],
                                    start=False, stop=lastr,
                                    skip_group_check=True)
                        if phase == 'mm':
                            continue
                        # Act: zp = prelu(z-halves); Act: xlg = copy(x-halves)
                        if act_prelu:
                            nc.scalar.activation(
                                out=zp[:, ch * HC:(ch + cnt) * HC],
                                in_=_b(pu[:], 0, [[256, cnt], [1, HC]]),
                                func=AF.Prelu, alpha=NEG)
                        else:
                            nc.vector.scalar_tensor_tensor(
                                out=zp[:, ch * HC:(ch + cnt) * HC],
                                in0=_b(pu[:], 0, [[256, cnt], [1, HC]]),
                                scalar=NEG,
                                in1=_b(pu[:], 0, [[256, cnt], [1, HC]]),
                                op0=AL.mult, op1=AL.max)
                        nc.scalar.copy(
                            out=xlg[:, ch * HC:(ch + cnt) * HC],
                            in_=_b(pu[:], HC, [[256, cnt], [1, HC]]))

                    if phase in ('mm', 'mmact'):
                        o2 = outp.tile([P, NBMAX * HC], f32, tag="o")
                        if phase == 'mm':
                            nc.vector.tensor_scalar_add(
                                out=o2[:, :1], in0=colt[:, :1], scalar1=1.0)
                        else:
                            nc.vector.tensor_scalar_add(
                                out=o2[:, :1], in0=zp[:, :1], scalar1=1.0)
                            nc.vector.tensor_scalar_add(
                                out=o2[:, 1:2], in0=xlg[:, :1], scalar1=1.0)
                        base = geo["starts"][t]
                        nc.sync.dma_start(
                            out=io["out"].ap()[base:base + 1, :].rearrange(
                                "r c -> r c"),
                            in_=o2[:1, :HC])
                        continue
                    # z *= att (bf16 packed 2x)
                    nc.vector.tensor_tensor(
                        out=zp[:, :FW], in0=zp[:, :FW],
                        in1=_b(ct["attb"][:], 0, [[0, NW], [1, HC]]),
                        op=AL.mult)
                    # treeC over c (c-major: stride H, chunks of H contiguous)
                    L = C
                    while L > 2:
                        half = L // 2
                        nc.vector.tensor_tensor(
                            out=_b(zp[:], 0, [[HC, NW], [H, half], [1, H]]),
                            in0=_b(zp[:], 0, [[HC, NW], [H, half], [1, H]]),
                            in1=_b(zp[:], half * H,
                                   [[HC, NW], [H, half], [1, H]]),
                            op=AL.add)
                        L = half
                    # final: logits[p, (nb, w, h)] fp32 = c0 + c1 + mask
                    lg = smp.tile([P, NWMAX * H], f32, tag="lg")
                    nc.vector.tensor_tensor(
                        out=_b(lg[:], 0, [[H, NW], [1, H]]),
                        in0=_b(zp[:], 0, [[HC, NW], [1, H]]),
                        in1=_b(zp[:], H, [[HC, NW], [1, H]]),
                        op=AL.add)
                    nc.vector.tensor_tensor(
                        out=lg[:, :NW * H], in0=lg[:, :NW * H],
                        in1=_b(auxt[:], mko, [[1, NW], [0, H]]),
                        op=AL.add)
                    # exp
                    nc.scalar.activation(
                        out=lg[:, :NW * H], in_=lg[:, :NW * H], func=AF.Exp)
                    # den over w; +eps; recip
                    den = smp.tile([P, NBMAX * H], f32, tag="den")
                    nc.vector.tensor_reduce(
                        out=den[:, :NB * H],
                        in_=_b(lg[:], 0, [[W * H, NB], [1, H], [H, W]]),
                        axis=AX.X, op=AL.add)
                    nc.vector.tensor_scalar_add(
                        out=den[:, :NB * H], in0=den[:, :NB * H],
                        scalar1=1e-16)
                    nc.vector.reciprocal(
                        out=den[:, :NB * H], in_=den[:, :NB * H])
                    # alpha (bf16) = ex * recip  [p, (nb, w, h)]
                    al = smp.tile([P, NWMAX * H], bf16, tag="al")
                    nc.vector.tensor_tensor(
                        out=al[:, :NW * H], in0=lg[:, :NW * H],
                        in1=_b(den[:], 0, [[H, NB], [0, W], [1, H]]),
                        op=AL.mult)
                    # msg = xlg * alpha (c-major: alpha [.., 0C, 1H] packed)
                    nc.vector.tensor_tensor(
                        out=xlg[:, :FW], in0=xlg[:, :FW],
                        in1=_b(al[:], 0, [[H, NW], [0, C], [1, H]]),
                        op=AL.mult)
                    # treeW over w (bf16 packed)
                    L = W
                    while L > 1:
                        half = (L + 1) // 2
                        k = L - half
                        nc.vector.tensor_tensor(
                            out=_b(xlg[:], 0, [[W * HC, NB], [HC, k], [1, HC]]),
                            in0=_b(xlg[:], 0, [[W * HC, NB], [HC, k], [1, HC]]),
                            in1=_b(xlg[:], half * HC,
                                   [[W * HC, NB], [HC, k], [1, HC]]),
                            op=AL.add)
                        L = half
                    o2 = outp.tile([P, NBMAX * HC], f32, tag="o")
                    if zero_bias:
                        nc.vector.tensor_scalar_add(
                            out=o2[:, :NB * HC],
                            in0=_b(xlg[:], 0, [[W * HC, NB], [1, HC]]),
                            scalar1=0.0)
                    else:
                        nc.vector.tensor_tensor(
                            out=o2[:, :NB * HC],
                            in0=_b(xlg[:], 0, [[W * HC, NB], [1, HC]]),
                            in1=_b(ct["bb"][:], 0, [[0, NB], [1, HC]]),
                            op=AL.add)
                    # ELU = relu(x) + exp(-relu(-x)) - 1
                    rt = outp.tile([P, NBMAX * HC], f32, tag="relu")
                    nc.scalar.activation(
                        out=rt[:, :NB * HC], in_=o2[:, :NB * HC], func=AF.Relu)
                    nc.scalar.activation(
                        out=o2[:, :NB * HC], in_=o2[:, :NB * HC],
                        func=AF.Relu, scale=-1.0)
                    nc.scalar.activation(
                        out=o2[:, :NB * HC], in_=o2[:, :NB * HC],
                        func=AF.Exp, scale=-1.0)
                    nc.vector.scalar_tensor_tensor(
                        out=o2[:, :NB * HC], in0=o2[:, :NB * HC],
                        scalar=-1.0, in1=rt[:, :NB * HC],
                        op0=AL.add, op1=AL.add)
                    base = geo["starts"][t]
                    nc.sync.dma_start(
                        out=io["out"].ap()[base:base + P * NB, :].rearrange(
                            "(p nb) c -> p nb c", p=P),
                        in_=_b(o2[:], 0, [[HC, NB], [1, HC]]))
            _ls.close()

    nc.compile()
    _dedupe_ldweights(nc.m)
    return nc


def _elu(x):
    return np.where(x > 0, x, np.expm1(np.minimum(x, 0.0))).astype(np.float32)


def _prep_all(inputs, n_cores):
    s = np.asarray(inputs['s'], np.float32)
    t = np.asarray(inputs['t'], np.float32)
    edges = np.asarray(inputs['edges'])
    ew = np.asarray(inputs['edge_weight'], np.float32)[:, 0]
    src_g, dst_g = edges[0].astype(np.int64), edges[1].astype(np.int64)
    n_nodes = s.shape[0]

    geo_s = _encoder_prep(n_nodes, s, t, src_g, dst_g, ew, n_cores)
    geo_t = _encoder_prep(n_nodes, t, s, dst_g, src_g, ew, n_cores)

    def bc(v, pdim=P):
        return np.broadcast_to(
            np.asarray(v, np.float32).reshape(-1)[_OLD_OF_NEW].astype(BF),
            (pdim, HC)).copy()

    consts = {}
    for e, sfx in enumerate("st"):
        wl = np.asarray(inputs[f"Wl_{sfx}"], np.float32)[:, _OLD_OF_NEW]
        wr = np.asarray(inputs[f"Wr_{sfx}"], np.float32)[:, _OLD_OF_NEW]
        consts[f"wl2_{sfx}"] = np.concatenate([wl, wl], axis=1).astype(BF)
        consts[f"wr2_{sfx}"] = np.concatenate([wr, wr], axis=1).astype(BF)
        bl = np.asarray(inputs[f"bl_{sfx}"], np.float32).reshape(-1)
        br = np.asarray(inputs[f"br_{sfx}"], np.float32).reshape(-1)
        we = np.asarray(inputs[f"We_{sfx}"], np.float32)[0].reshape(-1)
        web2 = np.stack([we, bl + br])[:, _OLD_OF_NEW].astype(BF)
        blb2 = np.stack([np.zeros(HC, np.float32), bl])[:, _OLD_OF_NEW].astype(BF)
        consts[f"web_{sfx}"] = web2
        consts[f"blb_{sfx}"] = blb2
        consts[f"attb_{sfx}"] = bc(inputs[f"att_{sfx}"])
        consts[f"bb_{sfx}"] = bc(inputs[f"b_{sfx}"])

    in_maps = []
    for c in range(n_cores):
        m = dict(
            aux_s=np.ascontiguousarray(geo_s["aux"][c]),
            aux_t=np.ascontiguousarray(geo_t["aux"][c]),
            ewT_s=geo_s["ewT"][c], ewT_t=geo_t["ewT"][c],
        )
        m.update(consts)
        in_maps.append(m)
    return geo_s, geo_t, in_maps


_CACHE = {}


def _get_program(inputs, n_cores=NCORES, act_prelu=True, loop_reps=1,
                 phase='all', **_ignored):
    geo_s, geo_t, in_maps = _prep_all(inputs, n_cores)
    zb = all(
        not np.any(np.asarray(inputs[f"{nm}_{sfx}"]))
        for nm in ("bl", "br", "b") for sfx in "st")
    key = (n_cores, zb, act_prelu, loop_reps, phase,
           tuple(geo_s["Ws"]), tuple(geo_s["NBs"]),
           tuple(geo_t["Ws"]), tuple(geo_t["NBs"]))
    if key not in _CACHE:
        _patch_walrus()
        nc = _build_program([geo_s, geo_t], zero_bias=zb,
                            act_prelu=act_prelu, loop_reps=loop_reps,
                            phase=phase)
        _CACHE[key] = nc
    return _CACHE[key], geo_s, geo_t, in_maps


def _unpermute(inputs, geo_s, geo_t, results, n_cores):
    n_nodes = np.asarray(inputs['s']).shape[0]
    outs = []
    for geo, sfx, bias in (
            (geo_s, "s", inputs["b_s"]), (geo_t, "t", inputs["b_t"])):
        full = np.tile(_elu(np.asarray(bias, np.float32)).reshape(1, HC),
                       (n_nodes, 1))
        for c in range(n_cores):
            nl = geo["node_lists"][c]
            # results are c-major; un-permute channels
            full[nl] = results[c][f"out_{sfx}"][:len(nl)][:, _NEW_OF_OLD]
        outs.append(full)
    return tuple(outs)


def kernel(**inputs):
    from concourse.bass_interp import get_hw_module
    from concourse import bass_utils
    _patch_walrus()
    nc, geo_s, geo_t, in_maps = _get_program(inputs)
    old_m = nc.m
    nc.m = get_hw_module(nc.m)
    try:
        res = bass_utils.run_bass_kernel_spmd(
            nc, in_maps, core_ids=list(range(NCORES)))
    finally:
        nc.m = old_m
    return _unpermute(inputs, geo_s, geo_t, res.results, NCORES)
